# revision 1
# baseline (speedup 1.0000x reference)
"""Trainium2 Bass kernel for nn_MultiHeadAttention_43971875177057.

MHA with residual: B=2, S=4096, d_model=512, n_heads=8, dk=64.
out = (softmax(mask(QK^T/sqrt(dk))) @ V).reshape(b,s,d) @ Wo^T + bo + x
where the reshape interleaves heads and sequence (no transpose back).

Sharding: 8 cores = (batch b in {0,1}) x (head-pair hp in {0..3}).
Each core computes attention for 2 heads of one batch over the full
sequence.  Key facts exploited:

 * The "faithful" reshape maps ctx[b,h,s,c] -> out row h*(S/8) + s//8,
   col (s%8)*64 + c.  So output rows [hp*1024, (hp+1)*1024) of batch b
   depend ONLY on heads 2hp, 2hp+1 of batch b -> clean row sharding.
 * Scores are computed TRANSPOSED (S^T[k,q]) so the attn weights come
   out of the PE with partition=k, which is exactly what the
   ctx^T = V^T @ attn^T matmul needs - no on-chip transposes at all.
 * Softmax needs no max-subtraction (scores ~ N(0,1), |s| < ~10) and
   the row-sum falls out of the ctx matmul for free via a ones-column
   appended to V.  Masking is multiplicative post-exp (exp(s)*m) which
   matches the reference's -1e9 additive mask exactly (both give 0.0
   for masked weights in fp32).
 * Q/K/V projections for quarters 1..3 are woven just-in-time into the
   first attention q-chunk so the PE never sits behind a serial
   prologue.
"""

import os
import sys
import types

import numpy as np
import ml_dtypes

B, S, D, H, DK = 2, 4096, 512, 8, 64
QC = 1024          # q-chunk (free dim of score tiles)
RT = S // 8        # output rows per head (the interleaved reshape)
BF16 = ml_dtypes.bfloat16


def _build_kernel(n_cores=8):
    import concourse.bacc as bacc
    import concourse.mybir as mybir
    import concourse.tile as tile
    import concourse.bass as bass

    f32 = mybir.dt.float32
    bf16 = mybir.dt.bfloat16
    C = D // 128       # 4 contraction chunks for the projections
    NKT = S // 128     # key tiles
    NQC = S // QC      # q chunks
    NKQ = NKT // NQC   # key tiles per quarter

    nc = bacc.Bacc("TRN2", target_bir_lowering=False, debug=False,
                   num_devices=n_cores)

    xT = nc.dram_tensor("xT", [D, S], bf16, kind="ExternalInput").ap()
    maskT = nc.dram_tensor("maskT", [S, S], bf16, kind="ExternalInput").ap()
    wq = nc.dram_tensor("wq", [128, C * 128], bf16, kind="ExternalInput").ap()
    wk = nc.dram_tensor("wk", [128, C * 128], bf16, kind="ExternalInput").ap()
    wv = nc.dram_tensor("wv", [128, C * 130], bf16, kind="ExternalInput").ap()
    wo = nc.dram_tensor("wo", [64, 8 * D], bf16, kind="ExternalInput").ap()
    bqs = nc.dram_tensor("bqs", [128, 1], f32, kind="ExternalInput").ap()
    bks = nc.dram_tensor("bks", [128, 1], f32, kind="ExternalInput").ap()
    bv = nc.dram_tensor("bv", [1, 130], f32, kind="ExternalInput").ap()
    bo = nc.dram_tensor("bo", [1, D], f32, kind="ExternalInput").ap()
    xres = nc.dram_tensor("xres", [2 * RT, D], f32, kind="ExternalInput").ap()
    out = nc.dram_tensor("out", [2 * RT, D], f32, kind="ExternalOutput").ap()
    rc_dram = nc.dram_tensor("rc_scratch", [2 * NQC, QC], f32).ap()
    sum_dram = nc.dram_tensor("sum_scratch", [2 * NQC, QC], f32).ap()

    Exp = mybir.ActivationFunctionType.Exp
    Mul = mybir.AluOpType.mult
    Add = mybir.AluOpType.add

    def pbcast(ap, p):
        # broadcast a [1, ...] DRAM AP along partitions
        return bass.AP(tensor=ap.tensor, offset=ap.offset,
                       ap=[[0, p]] + list(ap.ap[1:]))

    def mm(out_ps, lhsT, rhs, start, stop, width=512):
        # matmul split into <=512-wide sub-matmuls (one PSUM bank each)
        n = rhs.shape[-1]
        for i in range(0, n, width):
            w = min(width, n - i)
            nc.tensor.matmul(out_ps[:, i:i + w], lhsT=lhsT,
                             rhs=rhs[:, i:i + w], start=start, stop=stop)

    with tile.TileContext(nc) as tc:
        with (
            tc.tile_pool(name="const", bufs=1) as const,
            tc.tile_pool(name="mask", bufs=8) as maskp,
            tc.tile_pool(name="attn", bufs=8) as attnp,
            tc.tile_pool(name="outp", bufs=2) as outp,
            tc.tile_pool(name="small", bufs=2) as small,
            tc.tile_pool(name="psum", bufs=1, space="PSUM") as psum,
        ):
            # ---- loads ------------------------------------------------
            xT_r = xT.rearrange("(c p) s -> c p s", p=128)
            xt_c = []
            for c in range(C):
                t = const.tile([128, S], bf16, tag=f"xt{c}", name=f"xt{c}")
                nc.sync.dma_start(out=t, in_=xT_r[c])
                xt_c.append(t)
            wq_sb = const.tile([128, C, 128], bf16)
            nc.sync.dma_start(out=wq_sb, in_=wq.rearrange("p (c n) -> p c n", c=C))
            wk_sb = const.tile([128, C, 128], bf16)
            nc.sync.dma_start(out=wk_sb, in_=wk.rearrange("p (c n) -> p c n", c=C))
            wv_sb = const.tile([128, C, 130], bf16)
            nc.sync.dma_start(out=wv_sb, in_=wv.rearrange("p (c n) -> p c n", c=C))
            wo_sb = const.tile([64, 8, D], bf16)
            nc.sync.dma_start(out=wo_sb, in_=wo.rearrange("c (j f) -> c j f", j=8))
            bq_sb = const.tile([128, 1], f32)
            nc.sync.dma_start(out=bq_sb, in_=bqs)
            bk_sb = const.tile([128, 1], f32)
            nc.sync.dma_start(out=bk_sb, in_=bks)
            bv_sb = const.tile([128, 130], f32)
            nc.sync.dma_start(out=bv_sb, in_=pbcast(bv, 128))
            bo_sb = const.tile([128, D], f32)
            nc.sync.dma_start(out=bo_sb, in_=pbcast(bo, 128))

            # ---- projection producers --------------------------------
            qt_c = [None] * NQC
            kt_c = [None] * NQC
            v_c = [const.tile([128, NKQ, 130], bf16, tag=f"v{i}", name=f"v{i}")
                   for i in range(NQC)]

            def proj_qk(nm, w_sb, b_sb, lst, scale, i):
                ps = psum.tile([128, QC], f32, tag="s0" if nm == "qt" else "s1",
                               name="pqk")
                for c in range(C):
                    mm(ps, w_sb[:, c, :], xt_c[c][:, i * QC:(i + 1) * QC],
                       start=(c == 0), stop=(c == C - 1))
                t = const.tile([128, QC], bf16, tag=f"{nm}{i}", name=f"{nm}{i}")
                nc.vector.tensor_scalar(t, ps, scale, b_sb, Mul, Add)
                lst[i] = t

            def proj_v(kt):
                ps = psum.tile([128, 130], f32, tag="s1", name="pv")
                for c in range(C):
                    nc.tensor.matmul(ps, lhsT=xt_c[c][:, kt * 128:(kt + 1) * 128],
                                     rhs=wv_sb[:, c, :],
                                     start=(c == 0), stop=(c == C - 1))
                nc.vector.tensor_add(v_c[kt // NKQ][:, kt % NKQ, :], ps, bv_sb)

            def produce_quarter(i):
                proj_qk("qt", wq_sb, bq_sb, qt_c, 0.125, i)
                proj_qk("kt", wk_sb, bk_sb, kt_c, 1.0, i)
                for kt in range(i * NKQ, (i + 1) * NKQ):
                    proj_v(kt)

            for i in range(NQC):
                produce_quarter(i)

            # ---- attention --------------------------------------------
            ctxT = [const.tile([64, S], bf16, tag=f"ctxT{h}", name=f"ctxT{h}")
                    for h in (0, 1)]
            for qc in range(NQC):
                q0 = qc * QC
                ctx_ps = [psum.tile([65, QC], f32, tag=f"ctx{h}", name=f"ctx{h}")
                          for h in (0, 1)]
                for kt in range(NKT):
                    k0 = kt * 128
                    mt = maskp.tile([128, QC], bf16)
                    nc.gpsimd.dma_start(out=mt,
                                        in_=maskT[k0:k0 + 128, q0:q0 + QC])
                    for h in (0, 1):
                        sps = psum.tile([128, QC], f32, tag=f"s{h}")
                        kq = kt_c[k0 // QC]
                        kk = k0 % QC
                        mm(sps, kq[h * 64:(h + 1) * 64, kk:kk + 128],
                           qt_c[qc][h * 64:(h + 1) * 64, :],
                           start=True, stop=True)
                        at = attnp.tile([128, QC], bf16, tag=f"a{h}")
                        nc.scalar.activation(at, sps, Exp)
                        nc.vector.tensor_mul(at, at, mt)
                        mm(ctx_ps[h], v_c[kt // NKQ][:, kt % NKQ,
                                          h * 65:(h + 1) * 65], at,
                           start=(kt == 0), stop=(kt == NKT - 1))
                for h in (0, 1):
                    # 128-lane-parallel reciprocal of the softmax sums, then
                    # DRAM-bounce to a [64, QC] partition-broadcast.
                    srow = small.tile([1, QC], f32, tag="srow")
                    nc.vector.tensor_copy(srow, ctx_ps[h][64:65, :])
                    sraw = sum_dram[qc * 2 + h:qc * 2 + h + 1, :]
                    nc.sync.dma_start(out=sraw, in_=srow)
                    sums = small.tile([128, QC // 128], f32, tag="sums")
                    nc.sync.dma_start(
                        out=sums,
                        in_=sraw.rearrange("o (p f) -> (o p) f", p=128))
                    rc = small.tile([128, QC // 128], f32, tag="rc")
                    nc.vector.reciprocal(rc, sums)
                    row = rc_dram[qc * 2 + h:qc * 2 + h + 1, :]
                    nc.sync.dma_start(
                        out=row.rearrange("o (p f) -> (o p) f", p=128), in_=rc)
                    rcr = small.tile([64, QC], f32, tag="rcr")
                    nc.sync.dma_start(out=rcr, in_=pbcast(row, 64))
                    nc.vector.tensor_mul(ctxT[h][:, q0:q0 + QC],
                                         ctx_ps[h][0:64, :], rcr)

                # ---- output projection for this q-chunk ----------------
                rr = QC // 8
                for h in (0, 1):
                    ctx3 = ctxT[h].rearrange("p (t j) -> p j t", j=8)
                    r0 = h * RT + qc * rr
                    ops = psum.tile([128, D], f32, tag=f"ctx{h}", name=f"ops{h}")
                    for j in range(8):
                        nc.tensor.matmul(ops[:rr],
                                         lhsT=ctx3[:, j, qc * rr:(qc + 1) * rr],
                                         rhs=wo_sb[:, j, :],
                                         start=(j == 0), stop=(j == 7))
                    osb = outp.tile([128, D], f32, tag="osb")
                    nc.vector.tensor_add(osb[:rr], ops[:rr], bo_sb[:rr])
                    xr = outp.tile([128, D], f32, tag="xr")
                    nc.sync.dma_start(out=xr[:rr], in_=xres[r0:r0 + rr, :])
                    nc.vector.tensor_add(osb[:rr], osb[:rr], xr[:rr])
                    nc.sync.dma_start(out=out[r0:r0 + rr, :], in_=osb[:rr])

    nc.compile()
    return nc


def _shard_inputs(x, mask, Wq, bq, Wk, bk, Wv, bv, Wo, bo):
    """Host-side marshaling: slice/transpose/cast per core. core = b*4+hp."""
    C = D // 128
    keepT = np.ascontiguousarray((1 - mask[0, 0]).T).astype(BF16)
    woT = Wo.T.astype(np.float32)
    wo_re = np.ascontiguousarray(
        woT.reshape(8, 64, D).transpose(1, 0, 2).reshape(64, 8 * D)).astype(BF16)
    bo_re = bo.reshape(1, D).astype(np.float32)

    def re_w(wT):
        # [D, n] -> [128, C*n]  with  out[p, c*n+j] = wT[c*128+p, j]
        n = wT.shape[1]
        return np.ascontiguousarray(
            wT.reshape(C, 128, n).transpose(1, 0, 2).reshape(128, C * n)
        ).astype(BF16)

    in_maps = []
    for core in range(8):
        b, hp = divmod(core, 4)
        c0 = hp * 128
        wvT_ext = np.zeros((D, 130), np.float32)
        wvT_ext[:, 0:64] = Wv[c0:c0 + 64, :].T
        wvT_ext[:, 65:129] = Wv[c0 + 64:c0 + 128, :].T
        bv_ext = np.zeros((1, 130), np.float32)
        bv_ext[0, 0:64] = bv[c0:c0 + 64]
        bv_ext[0, 64] = 1.0
        bv_ext[0, 65:129] = bv[c0 + 64:c0 + 128]
        bv_ext[0, 129] = 1.0
        in_maps.append({
            "xT": np.ascontiguousarray(x[b].T).astype(BF16),
            "maskT": keepT,
            "wq": re_w(np.ascontiguousarray(Wq[c0:c0 + 128, :].T)),
            "wk": re_w(np.ascontiguousarray(Wk[c0:c0 + 128, :].T)),
            "wv": re_w(wvT_ext),
            "wo": wo_re,
            "bqs": (bq[c0:c0 + 128] / 8.0).reshape(128, 1).astype(np.float32),
            "bks": bk[c0:c0 + 128].reshape(128, 1).astype(np.float32),
            "bv": bv_ext,
            "bo": bo_re,
            "xres": np.ascontiguousarray(x[b, hp * 2 * RT:(hp + 1) * 2 * RT, :]),
        })
    return in_maps


_RESULT_CACHE = {}


def _ensure_env():
    """Make concourse importable and register the NTFF profile hook."""
    for p in ("/root/.axon_site/_ro/trn_rl_repo", "/opt/trn_rl_repo"):
        if os.path.isdir(p) and p not in sys.path:
            sys.path.append(p)
    try:
        import antenv  # noqa: F401
        import antenv.axon_hooks  # noqa: F401
    except ImportError:
        try:
            import antenv
            mod = types.ModuleType("antenv.axon_hooks")
            _hook = [None]
            mod.set_axon_ntff_profile_hook = lambda h: _hook.__setitem__(0, h)
            mod.get_axon_ntff_profile_hook = lambda: _hook[0]
            sys.modules["antenv.axon_hooks"] = mod
            antenv.axon_hooks = mod
            from trn_agent_boot.trn_boot import _ntff_profile_via_ctypes
            so = "/opt/axon/libaxon_pjrt.so"
            if os.path.exists(so):
                mod.set_axon_ntff_profile_hook(_ntff_profile_via_ctypes(so))
        except Exception:
            pass


def kernel(x, mask, Wq, bq, Wk, bk, Wv, bv, Wo, bo, trace=False):
    _ensure_env()
    from concourse.bass_utils import run_bass_kernel_spmd

    x = np.asarray(x, np.float32)
    mask = np.asarray(mask)
    args = [np.asarray(a, np.float32) for a in (Wq, bq, Wk, bk, Wv, bv, Wo, bo)]
    nc = _RESULT_CACHE.get("nc")
    if nc is None:
        nc = _build_kernel()
        _RESULT_CACHE["nc"] = nc
    in_maps = _shard_inputs(x, mask, *args)
    res = run_bass_kernel_spmd(nc, in_maps, core_ids=list(range(8)),
                               trace=trace)
    _RESULT_CACHE["last_run"] = res
    out = np.empty((B, S, D), np.float32)
    for core in range(8):
        b, hp = divmod(core, 4)
        out[b, hp * 2 * RT:(hp + 1) * 2 * RT, :] = res.results[core]["out"]
    return out


if __name__ == "__main__":
    _ensure_env()
    nc = _build_kernel()
    print("kernel built + compiled OK")

